# revision 8
# baseline (speedup 1.0000x reference)
"""Trainium2 Bass kernel for nn_Attention (dense transformer MHA block).

Reference computation (per batch element b of 8):
    qkv = x @ w_qkv;  q,k,v split into 16 heads of dim 64
    out = softmax(q k^T / 8) v  (per head),  y = out @ w_proj + b_proj

Topology: SINGLE CORE, batch-sequential. On this axon-tunneled setup the
bass-exec dispatch cost is ~1.5 ms per device-call and calls serialize
across devices, so an 8-way SPMD launch pays ~12-18 ms of dispatch per
invocation while the per-core kernel itself is only ~0.25 ms. Running all
8 batch elements sequentially on one core costs ~2 ms of device time +
one dispatch — several times faster end-to-end. x and the weights are
cast to bf16 on the host (error budget: bf16 rounding adds ~2-4e-3 rms
vs the 2e-2 gate), which halves transfer bytes and lets the DMA xbar
transpose (16-bit only) produce xT straight from DRAM.

Per-batch dataflow (bf16 operands, fp32 PSUM accumulate):
  0. (once) weights are DMA'd into resident SBUF tiles: wqk [pair]
     [P, DT, 2, P], wv [grp, dt][P, 512], wp [P, DT, D]; bias broadcast
     to [P, D] f32; vaug ones-columns (softmax denominator) written once.
  1. xT [d, n] bf16 via 16 DMA xbar transposes [512, 128] -> [128, 512]
     straight from DRAM (no PE, no PSUM).
  2. v-pass chains (xT-stationary, w_v-moving) run inside pairs 0-1 as
     PE filler; results land in v_aug [n, 16*(64+1)] bf16 (ones column
     per head = integrated softmax denominator).
  3. per pair: qT,kT [c,n] bf16 via w-stationary / xT-moving chains.
     Both heads of the pair advance together through each j-tile: their
     S matmuls (K=64) are issued back-to-back as 64-row PE tiles
     (tile_position rows 0/64) so they run concurrently in the array;
     each S/P tile packs [head-even | head-odd] for one i-half, and the
     two i-half sweeps are serialized so PV only ever needs two PSUM
     accumulators.
  4. P = exp(S/8) on ScalarE (PSUM -> SBUF, bf16); PV chains accumulate
     outT[65, i] over j (v_aug-stationary, P-moving); row 64 is the
     softmax denominator. normalize: reciprocal (DVE) ->
     partition_broadcast (GpSimd) -> multiply (DVE) into attn [c,n] bf16.
  5. proj (bf16): attn-stationary @ w_proj-moving, bias added during
     eviction (DVE), per-half y DMA out (f32).

  Batches pipeline through the same pools; xT is double-buffered across
  batches so batch b+1's transposes overlap batch b's attention drain.
  PSUM budget: mm 2 + s 2x2 + pv 2 = 8 banks exactly.
"""

import numpy as np
import ml_dtypes
from contextlib import ExitStack

import concourse.bass as bass
import concourse.bacc as bacc
import concourse.mybir as mybir
from concourse import tile

F32 = mybir.dt.float32
BF16 = mybir.dt.bfloat16
EXPF = mybir.ActivationFunctionType.Exp

B = 8             # batch (all on one core)
N = 1024          # sequence length
D = 1024          # model dim
H = 16            # heads
HD = 64           # head dim
SCALE = HD ** -0.5
P = 128           # partitions
NT = N // P       # 8 n-tiles
DT = D // P       # 8 d-chunks


def _build(tc, nc, x_d, wqkv_d, wproj_d, bproj_d, y_d):
    mul = mybir.AluOpType.mult
    add = mybir.AluOpType.add

    with ExitStack() as outer:
        const = outer.enter_context(tc.tile_pool(name="const", bufs=1))
        ones16 = const.tile([P, H], BF16)
        bias_bc = const.tile([P, D], F32)

        # ---- resident weights (bf16, loaded once) ----
        wpool = outer.enter_context(tc.tile_pool(name="weights", bufs=1))

        nc.gpsimd.memset(ones16[:], 1.0)

        # q/k weights: per pair [P, DT, 2, P]
        wqk = []
        for pair in range(H // 2):
            wb = wpool.tile([P, DT, 2, P], BF16, tag=f"wqk{pair}",
                            name=f"wqk{pair}")
            src = wqkv_d.rearrange("(dt p) (k r e) -> p dt k r e", p=P, k=3,
                                   e=P)[:, :, 0:2, pair, :]
            for which in range(2):
                nc.sync.dma_start(wb[:, :, which, :], src[:, :, which, :])
            wqk.append(wb)

        # v weights: wv_r[(grp, dt)] -> [P, 512] slices
        wv_r = {}
        wsrc = wqkv_d.rearrange("(dt p) e -> p dt e", p=P)
        for grp in range(2):
            for half in range(2):
                wvt = wpool.tile([P, 4, 512], BF16, tag=f"wv{grp}h{half}",
                                 name=f"wv{grp}h{half}")
                nc.sync.dma_start(
                    wvt[:],
                    wsrc[:, half * 4:(half + 1) * 4,
                         2 * D + grp * 512: 2 * D + (grp + 1) * 512])
                for dt in range(4):
                    wv_r[(grp, half * 4 + dt)] = wvt[:, dt, :]

        # proj weights [P, DT, D]
        wp_t = wpool.tile([P, DT, D], BF16, tag="wp", name="wp_t")
        nc.sync.dma_start(
            wp_t[:], wproj_d.rearrange("(cc p) e -> p cc e", p=P))

        bst = const.tile([1, D], F32)
        nc.sync.dma_start(bst[:], bproj_d[:].rearrange("(a f) -> a f", a=1))
        nc.gpsimd.partition_broadcast(bias_bc[:], bst[:])

        # ---- per-batch pools (allocated once, reused each iteration) ----
        # xT double-buffered across batches: 16 tiles per generation.
        xt_pool = outer.enter_context(tc.tile_pool(name="xT", bufs=4 * DT))
        vaug_pool = outer.enter_context(tc.tile_pool(name="vaug", bufs=NT))
        attn_pool = outer.enter_context(tc.tile_pool(name="attnout", bufs=DT))
        outsb = outer.enter_context(tc.tile_pool(name="outsb", bufs=3))
        mm_ps = outer.enter_context(
            tc.tile_pool(name="mmps", bufs=2, space="PSUM"))
        qk_pool = outer.enter_context(tc.tile_pool(name="qk", bufs=4))
        p_pool = outer.enter_context(tc.tile_pool(name="pT", bufs=12))
        pv_ps = outer.enter_context(
            tc.tile_pool(name="pvps", bufs=2, space="PSUM"))
        s_ps = outer.enter_context(
            tc.tile_pool(name="sps", bufs=2, space="PSUM"))
        rt_pool = outer.enter_context(tc.tile_pool(name="rt", bufs=2))
        bt_pool = outer.enter_context(tc.tile_pool(name="bt", bufs=2))

        # vaug tiles persist across batches; ones columns written once.
        vaug = [vaug_pool.tile([P, H * (HD + 1)], BF16, tag="vaug",
                               name=f"vaug{i}") for i in range(NT)]
        for nt in range(NT):
            nc.vector.tensor_copy(
                vaug[nt][:].rearrange("p (h e) -> p h e", h=H)[:, :, HD:HD + 1],
                ones16[:].rearrange("p (h e) -> p h e", e=1))

        for b in range(B):
            _build_batch(tc, nc, b, x_d, y_d, wqk, wv_r, wp_t, bias_bc,
                         vaug, xt_pool, attn_pool, outsb, mm_ps, qk_pool,
                         p_pool, pv_ps, s_ps, rt_pool, bt_pool)


def _build_batch(tc, nc, b, x_d, y_d, wqk, wv_r, wp_t, bias_bc, vaug,
                 xt_pool, attn_pool, outsb, mm_ps, qk_pool, p_pool, pv_ps,
                 s_ps, rt_pool, bt_pool):
    mul = mybir.AluOpType.mult
    add = mybir.AluOpType.add
    x0 = b * N

    # xT half-tiles per d-chunk: n 0-511 and 512-1023 (dep tracking is
    # tile-granular; qk chains for one n-half only wait on that half).
    xT = [(xt_pool.tile([P, N // 2], BF16, tag="xT", name=f"xTa{b}_{i}"),
           xt_pool.tile([P, N // 2], BF16, tag="xT", name=f"xTb{b}_{i}"))
          for i in range(DT)]

    def xT_n(dt, nt):
        return xT[dt][nt // 4][:, (nt % 4) * P:(nt % 4 + 1) * P]

    attn_t = [attn_pool.tile([P, N], BF16, tag="attn", name=f"attn{b}_{i}")
              for i in range(DT)]

    # ---- phase A: xT via DMA xbar transpose straight from DRAM ----
    for dt in range(DT):
        for nch in range(2):
            nc.sync.dma_start_transpose(
                xT[dt][nch][:],
                x_d[x0 + nch * 512:x0 + (nch + 1) * 512,
                    dt * P:(dt + 1) * P])

    def v_chains(cv, nt_list):
        for nt in nt_list:
            vp = mm_ps.tile([P, 512], F32, tag="mm")
            for dt in range(DT):
                nc.tensor.matmul(
                    vp[:], xT_n(dt, nt),
                    wv_r[(cv, dt)], start=(dt == 0), stop=(dt == DT - 1))
            dst = vaug[nt][:].rearrange(
                "p (h e) -> p h e", h=H)[:, 8 * cv:8 * cv + 8, 0:HD]
            src = vp[:].rearrange("p (h e) -> p h e", h=8)
            nc.vector.tensor_copy(dst, src)

    # ---- phase B: per head pair qk + attention ----
    for pair in range(H // 2):
        wqk_r = wqk[pair]
        # per-(which, nch) tiles: tile-granular dep tracking lets S
        # matmuls start after half the qk chains
        qk_t = {}
        for which in range(2):   # 0 = q, 1 = k
            for nch in range(2):
                ct = qk_pool.tile([P, 512], BF16, tag="qk",
                                  name=f"qk{b}_{pair}_{which}{nch}")
                qp = mm_ps.tile([P, 512], F32, tag="mm")
                for dt in range(DT):
                    nc.tensor.matmul(
                        qp[:], wqk_r[:, dt, which, :],
                        xT[dt][nch][:],
                        start=(dt == 0), stop=(dt == DT - 1))
                nc.vector.tensor_copy(ct[:], qp[:])
                qk_t[(which, nch)] = ct

        if pair == 0:
            # cv0 feeds heads 0-7: must precede pair 0's heads
            v_chains(0, range(NT))
        # Both heads of the pair advance together: their S matmuls are
        # 64-row PE tiles (rows 0-63 / 64-127) issued back-to-back, so
        # they run concurrently in the array. Each s/p tile packs
        # [head-even | head-odd] for one ich half; the two ich sweeps
        # are serialized so PV only needs two PSUM accumulators.
        h_e, h_o = 2 * pair, 2 * pair + 1
        for ich in range(2):
            sl = slice(ich * 512, (ich + 1) * 512)
            pv_e = pv_ps.tile([HD + 1, 512], F32, tag="pv",
                              name=f"pv{b}_{h_e}_{ich}")
            pv_o = pv_ps.tile([HD + 1, 512], F32, tag="pv",
                              name=f"pv{b}_{h_o}_{ich}")
            for jtb in range(0, NT, 2):
                sps, pts = [], []
                for jt in (jtb, jtb + 1):
                    sp = s_ps.tile([P, N], F32, tag="s")
                    kt_ = qk_t[(1, jt // 4)]
                    kc = (jt % 4) * P
                    qt_ = qk_t[(0, ich)]
                    nc.tensor.matmul(
                        sp[:, 0:512],
                        kt_[0:HD, kc:kc + P],
                        qt_[0:HD, :],
                        start=True, stop=True,
                        tile_position=(0, 0))
                    nc.tensor.matmul(
                        sp[:, 512:1024],
                        kt_[HD:P, kc:kc + P],
                        qt_[HD:P, :],
                        start=True, stop=True,
                        tile_position=(64, 0))
                    sps.append(sp)
                for jt in (jtb, jtb + 1):
                    pt = p_pool.tile([P, N], BF16, tag="p",
                                     name=f"pT{b}_{pair}_{ich}_{jt}")
                    nc.scalar.activation(pt[:], sps[jt - jtb][:],
                                         EXPF, scale=SCALE)
                    pts.append(pt)
                for jt in (jtb, jtb + 1):
                    pt = pts[jt - jtb]
                    nc.tensor.matmul(
                        pv_e[:],
                        vaug[jt][:, h_e * (HD + 1):(h_e + 1) * (HD + 1)],
                        pt[:, 0:512],
                        start=(jt == 0), stop=(jt == NT - 1))
                    nc.tensor.matmul(
                        pv_o[:],
                        vaug[jt][:, h_o * (HD + 1):(h_o + 1) * (HD + 1)],
                        pt[:, 512:1024],
                        start=(jt == 0), stop=(jt == NT - 1))
            for hh, pvt in ((0, pv_e), (1, pv_o)):
                base = HD * hh
                rt = rt_pool.tile([1, 512], F32, tag="rt",
                                  name=f"rt{b}_{pair}_{hh}_{ich}")
                bt = bt_pool.tile([HD, 512], F32, tag="bt",
                                  name=f"bt{b}_{pair}_{hh}_{ich}")
                nc.vector.reciprocal(rt[:], pvt[HD:HD + 1, :])
                nc.gpsimd.partition_broadcast(bt[:], rt[:])
                nc.vector.tensor_tensor(
                    attn_t[pair][base:base + HD, sl],
                    pvt[0:HD, :], bt[:], mul)

        if pair == 1:
            # cv1 feeds heads 8-15 (pairs 4-7): post-heads PE filler
            # under pair 1's exp stream
            v_chains(1, range(NT))

    # ---- projection ----
    for nt in range(NT):
        yo = outsb.tile([P, D], F32, tag="y", name=f"yo{b}_{nt}")
        for ec in range(2):
            pool_ = mm_ps if (2 * nt + ec) % 3 == 2 else pv_ps
            yp = pool_.tile([P, 512], F32,
                            tag="mm" if pool_ is mm_ps else "pv")
            for cc in range(DT):
                nc.tensor.matmul(
                    yp[:], attn_t[cc][:, nt * P:(nt + 1) * P],
                    wp_t[:, cc, ec * 512:(ec + 1) * 512],
                    start=(cc == 0), stop=(cc == DT - 1))
            nc.vector.tensor_tensor(
                yo[:, ec * 512:(ec + 1) * 512], yp[:],
                bias_bc[:, ec * 512:(ec + 1) * 512], add)
            nc.sync.dma_start(
                y_d[x0 + nt * P:x0 + (nt + 1) * P,
                    ec * 512:(ec + 1) * 512],
                yo[:, ec * 512:(ec + 1) * 512])


def build_nc():
    nc = bacc.Bacc("TRN2", target_bir_lowering=False, debug=False)
    x_d = nc.dram_tensor("x", [B * N, D], BF16, kind="ExternalInput").ap()
    wqkv_d = nc.dram_tensor("w_qkv", [D, 3 * D], BF16, kind="ExternalInput").ap()
    wproj_d = nc.dram_tensor("w_proj", [D, D], BF16, kind="ExternalInput").ap()
    bproj_d = nc.dram_tensor("b_proj", [D], F32, kind="ExternalInput").ap()
    y_d = nc.dram_tensor("y", [B * N, D], F32, kind="ExternalOutput").ap()
    with tile.TileContext(nc) as tc:
        _build(tc, nc, x_d, wqkv_d, wproj_d, bproj_d, y_d)
    nc.compile()
    return nc


_NC = None
_EXEC = None     # (compiled_fn, in_names, in_dtypes, zero_outs)


def get_exec():
    """Build (once) a cached single-device jit callable for the NEFF.

    run_bass_via_pjrt re-traces a fresh jit closure on every call; caching
    the compiled callable makes repeat kernel() invocations cheap.
    """
    global _NC, _EXEC
    if _EXEC is not None:
        return _EXEC
    if _NC is None:
        _NC = build_nc()
    nc = _NC
    import jax
    from concourse import bass2jax as b2j

    b2j.install_neuronx_cc_hook()
    pname = nc.partition_id_tensor.name if nc.partition_id_tensor else None
    in_names, in_dtypes, out_names, out_avals, zero_outs = [], [], [], [], []
    for alloc in nc.m.functions[0].allocations:
        if not isinstance(alloc, mybir.MemoryLocationSet):
            continue
        name = alloc.memorylocations[0].name
        if alloc.kind == "ExternalInput":
            if name != pname:
                in_names.append(name)
                in_dtypes.append(mybir.dt.np(alloc.dtype))
        elif alloc.kind == "ExternalOutput":
            out_names.append(name)
            shape = tuple(alloc.tensor_shape)
            dtype = mybir.dt.np(alloc.dtype)
            out_avals.append(jax.core.ShapedArray(shape, dtype))
            zero_outs.append(np.zeros(shape, dtype))
    all_names = in_names + out_names
    if pname is not None:
        all_names = all_names + [pname]

    def _bodyfn(*args):
        operands = list(args)
        if pname is not None:
            operands.append(b2j.partition_id_tensor())
        return tuple(b2j._bass_exec_p.bind(
            *operands, out_avals=tuple(out_avals), in_names=tuple(all_names),
            out_names=tuple(out_names), lowering_input_output_aliases=(),
            sim_require_finite=True, sim_require_nnan=True, nc=nc))

    fn = jax.jit(_bodyfn)
    _EXEC = (fn, in_names, in_dtypes, zero_outs)
    return _EXEC


def prep_inputs(x, w_qkv, w_proj, b_proj):
    bf = ml_dtypes.bfloat16
    return {
        "x": np.ascontiguousarray(
            np.asarray(x, dtype=np.float32).astype(bf)).reshape(B * N, D),
        "w_qkv": np.ascontiguousarray(
            np.asarray(w_qkv, dtype=np.float32).astype(bf)),
        "w_proj": np.ascontiguousarray(
            np.asarray(w_proj, dtype=np.float32).astype(bf)),
        "b_proj": np.ascontiguousarray(np.asarray(b_proj, dtype=np.float32)),
    }


def kernel(x, w_qkv, w_proj, b_proj):
    import jax
    fn, in_names, in_dtypes, zero_outs = get_exec()
    host = prep_inputs(x, w_qkv, w_proj, b_proj)
    args = [np.asarray(host[n_], dtype=dt_) for n_, dt_ in
            zip(in_names, in_dtypes)] + [z.copy() for z in zero_outs]
    outs = fn(*args)
    y = np.asarray(jax.block_until_ready(outs)[0], dtype=np.float32)
    return y.reshape(B, N, D)


# revision 20
# speedup vs baseline: 1.6498x; 1.6498x over previous
"""Trainium2 Bass kernel for nn_Attention (dense transformer MHA block).

Reference computation (per batch element b of 8):
    qkv = x @ w_qkv;  q,k,v split into 16 heads of dim 64
    out = softmax(q k^T / 8) v  (per head),  y = out @ w_proj + b_proj

Topology: SINGLE CORE, batch-sequential. On this axon-tunneled setup the
bass-exec dispatch cost is ~1.5 ms per device-call and calls serialize
across devices, so an 8-way SPMD launch pays ~12-18 ms of dispatch per
invocation while the per-core kernel itself is only ~0.25 ms. Running all
8 batch elements sequentially on one core costs ~2 ms of device time +
one dispatch — several times faster end-to-end. x and the weights are
cast to bf16 on the host (error budget: bf16 rounding adds ~2-4e-3 rms
vs the 2e-2 gate), which halves transfer bytes and lets the DMA xbar
transpose (16-bit only) produce xT straight from DRAM.

Per-batch dataflow (bf16 operands, fp32 PSUM accumulate):
  0. (once) weights are DMA'd into resident SBUF tiles: wqk [pair]
     [P, DT, 2, P], wv [grp, dt][P, 512], wp [P, DT, D]; bias broadcast
     to [P, D] f32; vaug ones-columns (softmax denominator) written once.
  1. xT [d, n] bf16 via 16 DMA xbar transposes [512, 128] -> [128, 512]
     straight from DRAM (no PE, no PSUM).
  2. v-pass chains (xT-stationary, w_v-moving) run inside pairs 0-1 as
     PE filler; results land in v_aug [n, 16*(64+1)] bf16 (ones column
     per head = integrated softmax denominator).
  3. per pair: qT,kT [c,n] bf16 via w-stationary / xT-moving chains.
     Both heads of the pair advance together through each j-tile: their
     S matmuls (K=64) are issued back-to-back as 64-row PE tiles
     (tile_position rows 0/64) so they run concurrently in the array;
     each S/P tile packs [head-even | head-odd] for one i-half, and the
     two i-half sweeps are serialized so PV only ever needs two PSUM
     accumulators.
  4. P = exp(S/8) on ScalarE (PSUM -> SBUF, bf16); PV chains accumulate
     outT[65, i] over j (v_aug-stationary, P-moving); row 64 is the
     softmax denominator. normalize: reciprocal (DVE) ->
     partition_broadcast (GpSimd) -> multiply (DVE) into attn [c,n] bf16.
  5. proj (bf16): attn-stationary @ w_proj-moving, bias added during
     eviction (DVE), per-half y DMA out (f32).

  Batches pipeline through the same pools; xT is double-buffered across
  batches so batch b+1's transposes overlap batch b's attention drain.
  PSUM budget: mm 2 + s 2x2 + pv 2 = 8 banks exactly.
"""

import numpy as np
import ml_dtypes
from contextlib import ExitStack

import concourse.bass as bass
import concourse.bacc as bacc
import concourse.mybir as mybir
from concourse import tile

F32 = mybir.dt.float32
BF16 = mybir.dt.bfloat16
EXPF = mybir.ActivationFunctionType.Exp

B = 8             # batch (all on one core)
N = 1024          # sequence length
D = 1024          # model dim
H = 16            # heads
HD = 64           # head dim
SCALE = HD ** -0.5
P = 128           # partitions
NT = N // P       # 8 n-tiles
DT = D // P       # 8 d-chunks


def _build(tc, nc, x_d, wqkv_d, wproj_d, bproj_d, y_d, repeat=1):
    mul = mybir.AluOpType.mult
    add = mybir.AluOpType.add

    with ExitStack() as outer:
        const = outer.enter_context(tc.tile_pool(name="const", bufs=1))
        ones16 = const.tile([P, H], BF16)
        bias_bc = const.tile([P, D], F32)

        # ---- resident weights (bf16, loaded once) ----
        wpool = outer.enter_context(tc.tile_pool(name="weights", bufs=1))

        nc.gpsimd.memset(ones16[:], 1.0)

        # q/k weights: per pair [P, DT, 2, P]. Pair 0 is DMA'd first, then
        # batch 0's xT transposes are issued (see below) so the first qk
        # chain isn't gated on the whole weight load.
        wqk = []
        wqk_src = wqkv_d.rearrange("(dt p) (k r e) -> p dt k r e", p=P, k=3,
                                   e=P)[:, :, 0:2, :, :]

        def load_wqk(pair):
            wb = wpool.tile([P, DT, 2, P], BF16, tag=f"wqk{pair}",
                            name=f"wqk{pair}")
            for which in range(2):
                nc.sync.dma_start(wb[:, :, which, :],
                                  wqk_src[:, :, which, pair, :])
            wqk.append(wb)

        load_wqk(0)

        # batch 0's xT, issued ahead of the remaining weight loads
        xt_pool = outer.enter_context(tc.tile_pool(name="xT", bufs=4 * DT))

        def issue_xT(it):
            b = it % B
            xT = [(xt_pool.tile([P, N // 2], BF16, tag="xT",
                                name=f"xTa{it}_{i}"),
                   xt_pool.tile([P, N // 2], BF16, tag="xT",
                                name=f"xTb{it}_{i}"))
                  for i in range(DT)]
            for dt in range(DT):
                for nch in range(2):
                    nc.sync.dma_start_transpose(
                        xT[dt][nch][:],
                        x_d[b * N + nch * 512:b * N + (nch + 1) * 512,
                            dt * P:(dt + 1) * P])
            return xT

        xT0 = issue_xT(0)

        for pair in range(1, H // 2):
            load_wqk(pair)

        # v weights: wv_r[(grp, dt)] -> [P, 512] slices
        wv_r = {}
        wsrc = wqkv_d.rearrange("(dt p) e -> p dt e", p=P)
        for grp in range(2):
            for half in range(2):
                wvt = wpool.tile([P, 4, 512], BF16, tag=f"wv{grp}h{half}",
                                 name=f"wv{grp}h{half}")
                nc.sync.dma_start(
                    wvt[:],
                    wsrc[:, half * 4:(half + 1) * 4,
                         2 * D + grp * 512: 2 * D + (grp + 1) * 512])
                for dt in range(4):
                    wv_r[(grp, half * 4 + dt)] = wvt[:, dt, :]

        # proj weights [P, DT, D]
        wp_t = wpool.tile([P, DT, D], BF16, tag="wp", name="wp_t")
        nc.sync.dma_start(
            wp_t[:], wproj_d.rearrange("(cc p) e -> p cc e", p=P))

        bst = const.tile([1, D], F32)
        nc.sync.dma_start(bst[:], bproj_d[:].rearrange("(a f) -> a f", a=1))
        nc.gpsimd.partition_broadcast(bias_bc[:], bst[:])

        # ---- per-batch pools (allocated once, reused each iteration) ----
        vaug_pool = outer.enter_context(tc.tile_pool(name="vaug", bufs=NT))
        attn_pool = outer.enter_context(tc.tile_pool(name="attnout", bufs=DT))
        outsb = outer.enter_context(tc.tile_pool(name="outsb", bufs=3))
        mm_ps = outer.enter_context(
            tc.tile_pool(name="mmps", bufs=2, space="PSUM"))
        qk_pool = outer.enter_context(tc.tile_pool(name="qk", bufs=4))
        p_pool = outer.enter_context(tc.tile_pool(name="pT", bufs=12))
        pv_ps = outer.enter_context(
            tc.tile_pool(name="pvps", bufs=2, space="PSUM"))
        s_ps = outer.enter_context(
            tc.tile_pool(name="sps", bufs=2, space="PSUM"))
        rt_pool = outer.enter_context(tc.tile_pool(name="rt", bufs=2))
        bt_pool = outer.enter_context(tc.tile_pool(name="bt", bufs=2))

        # vaug tiles persist across batches; ones columns written once.
        vaug = [vaug_pool.tile([P, H * (HD + 1)], BF16, tag="vaug",
                               name=f"vaug{i}") for i in range(NT)]
        for nt in range(NT):
            nc.vector.tensor_copy(
                vaug[nt][:].rearrange("p (h e) -> p h e", h=H)[:, :, HD:HD + 1],
                ones16[:].rearrange("p (h e) -> p h e", e=1))

        for it in range(repeat * B):
            xT = xT0 if it == 0 else issue_xT(it)
            _build_batch(tc, nc, it % B, it, x_d, y_d, xT, wqk, wv_r, wp_t,
                         bias_bc, vaug, attn_pool, outsb, mm_ps, qk_pool,
                         p_pool, pv_ps, s_ps, rt_pool, bt_pool)


def _build_batch(tc, nc, b, it, x_d, y_d, xT, wqk, wv_r, wp_t, bias_bc, vaug,
                 attn_pool, outsb, mm_ps, qk_pool, p_pool, pv_ps,
                 s_ps, rt_pool, bt_pool):
    mul = mybir.AluOpType.mult
    add = mybir.AluOpType.add
    x0 = b * N

    # xT half-tiles per d-chunk: n 0-511 and 512-1023 (dep tracking is
    # tile-granular; qk chains for one n-half only wait on that half).
    def xT_n(dt, nt):
        return xT[dt][nt // 4][:, (nt % 4) * P:(nt % 4 + 1) * P]

    attn_t = [attn_pool.tile([P, N], BF16, tag="attn", name=f"attn{it}_{i}")
              for i in range(DT)]

    def v_chains(cv, nt_list):
        for nt in nt_list:
            vp = mm_ps.tile([P, 512], F32, tag="mm")
            for dt in range(DT):
                nc.tensor.matmul(
                    vp[:], xT_n(dt, nt),
                    wv_r[(cv, dt)], start=(dt == 0), stop=(dt == DT - 1))
            dst = vaug[nt][:].rearrange(
                "p (h e) -> p h e", h=H)[:, 8 * cv:8 * cv + 8, 0:HD]
            src = vp[:].rearrange("p (h e) -> p h e", h=8)
            nc.vector.tensor_copy(dst, src)

    # ---- phase B: per head pair qk + attention ----
    for pair in range(H // 2):
        wqk_r = wqk[pair]
        # per-(which, nch) tiles: tile-granular dep tracking lets S
        # matmuls start after half the qk chains
        qk_t = {}
        for which in range(2):   # 0 = q, 1 = k
            for nch in range(2):
                ct = qk_pool.tile([P, 512], BF16, tag="qk",
                                  name=f"qk{it}_{pair}_{which}{nch}")
                qp = mm_ps.tile([P, 512], F32, tag="mm")
                for dt in range(DT):
                    nc.tensor.matmul(
                        qp[:], wqk_r[:, dt, which, :],
                        xT[dt][nch][:],
                        start=(dt == 0), stop=(dt == DT - 1))
                nc.vector.tensor_copy(ct[:], qp[:])
                qk_t[(which, nch)] = ct

        if pair == 0:
            # cv0 feeds heads 0-7: must precede pair 0's heads
            v_chains(0, range(NT))
        # Both heads of the pair advance together: their S matmuls are
        # 64-row PE tiles (rows 0-63 / 64-127) issued back-to-back, so
        # they run concurrently in the array. Each s/p tile packs
        # [head-even | head-odd] for one ich half; the two ich sweeps
        # are serialized so PV only needs two PSUM accumulators.
        h_e, h_o = 2 * pair, 2 * pair + 1
        for ich in range(2):
            sl = slice(ich * 512, (ich + 1) * 512)
            pv_e = pv_ps.tile([HD + 1, 512], F32, tag="pv",
                              name=f"pv{it}_{h_e}_{ich}")
            pv_o = pv_ps.tile([HD + 1, 512], F32, tag="pv",
                              name=f"pv{it}_{h_o}_{ich}")
            for jtb in range(0, NT, 2):
                sps, pts = [], []
                for jt in (jtb, jtb + 1):
                    sp = s_ps.tile([P, N], F32, tag="s")
                    kt_ = qk_t[(1, jt // 4)]
                    kc = (jt % 4) * P
                    qt_ = qk_t[(0, ich)]
                    nc.tensor.matmul(
                        sp[:, 0:512],
                        kt_[0:HD, kc:kc + P],
                        qt_[0:HD, :],
                        start=True, stop=True,
                        tile_position=(0, 0))
                    nc.tensor.matmul(
                        sp[:, 512:1024],
                        kt_[HD:P, kc:kc + P],
                        qt_[HD:P, :],
                        start=True, stop=True,
                        tile_position=(64, 0))
                    sps.append(sp)
                for jt in (jtb, jtb + 1):
                    pt = p_pool.tile([P, N], BF16, tag="p",
                                     name=f"pT{it}_{pair}_{ich}_{jt}")
                    nc.scalar.activation(pt[:], sps[jt - jtb][:],
                                         EXPF, scale=SCALE)
                    pts.append(pt)
                for jt in (jtb, jtb + 1):
                    pt = pts[jt - jtb]
                    nc.tensor.matmul(
                        pv_e[:],
                        vaug[jt][:, h_e * (HD + 1):(h_e + 1) * (HD + 1)],
                        pt[:, 0:512],
                        start=(jt == 0), stop=(jt == NT - 1))
                    nc.tensor.matmul(
                        pv_o[:],
                        vaug[jt][:, h_o * (HD + 1):(h_o + 1) * (HD + 1)],
                        pt[:, 512:1024],
                        start=(jt == 0), stop=(jt == NT - 1))
            for hh, pvt in ((0, pv_e), (1, pv_o)):
                base = HD * hh
                rt = rt_pool.tile([1, 512], F32, tag="rt",
                                  name=f"rt{it}_{pair}_{hh}_{ich}")
                bt = bt_pool.tile([HD, 512], F32, tag="bt",
                                  name=f"bt{it}_{pair}_{hh}_{ich}")
                nc.vector.reciprocal(rt[:], pvt[HD:HD + 1, :])
                nc.gpsimd.partition_broadcast(bt[:], rt[:])
                nc.vector.tensor_tensor(
                    attn_t[pair][base:base + HD, sl],
                    pvt[0:HD, :], bt[:], mul)

        if pair == 1:
            # cv1 feeds heads 8-15 (pairs 4-7): post-heads PE filler
            # under pair 1's exp stream
            v_chains(1, range(NT))

    # ---- projection ----
    for nt in range(NT):
        yo = outsb.tile([P, D], F32, tag="y", name=f"yo{it}_{nt}")
        for ec in range(2):
            # proj accumulators stay out of mm_ps so the next batch's qk
            # chains (which rotate mm_ps) don't WAR-stall on proj drain
            yp = pv_ps.tile([P, 512], F32, tag="pv")
            for cc in range(DT):
                nc.tensor.matmul(
                    yp[:], attn_t[cc][:, nt * P:(nt + 1) * P],
                    wp_t[:, cc, ec * 512:(ec + 1) * 512],
                    start=(cc == 0), stop=(cc == DT - 1))
            nc.vector.tensor_tensor(
                yo[:, ec * 512:(ec + 1) * 512], yp[:],
                bias_bc[:, ec * 512:(ec + 1) * 512], add)
            # y stores go out on the SWDGE (gpsimd) queue: they wait on proj
            # results, and on the in-order sync queue they would block the
            # next batch's xT transposes from starting.
            nc.gpsimd.dma_start(
                y_d[x0 + nt * P:x0 + (nt + 1) * P,
                    ec * 512:(ec + 1) * 512],
                yo[:, ec * 512:(ec + 1) * 512])


def build_nc(repeat=1):
    nc = bacc.Bacc("TRN2", target_bir_lowering=False, debug=False)
    x_d = nc.dram_tensor("x", [B * N, D], BF16, kind="ExternalInput").ap()
    wqkv_d = nc.dram_tensor("w_qkv", [D, 3 * D], BF16, kind="ExternalInput").ap()
    wproj_d = nc.dram_tensor("w_proj", [D, D], BF16, kind="ExternalInput").ap()
    bproj_d = nc.dram_tensor("b_proj", [D], F32, kind="ExternalInput").ap()
    y_d = nc.dram_tensor("y", [B * N, D], F32, kind="ExternalOutput").ap()
    with tile.TileContext(nc) as tc:
        _build(tc, nc, x_d, wqkv_d, wproj_d, bproj_d, y_d, repeat=repeat)
    nc.compile()
    return nc


_NC = None
_EXEC = None     # (compiled_fn, in_names, in_dtypes, zero_outs)


def _exec_parts(nc):
    import jax

    pname = nc.partition_id_tensor.name if nc.partition_id_tensor else None
    in_names, in_dtypes, out_names, out_avals, zero_outs = [], [], [], [], []
    for alloc in nc.m.functions[0].allocations:
        if not isinstance(alloc, mybir.MemoryLocationSet):
            continue
        name = alloc.memorylocations[0].name
        if alloc.kind == "ExternalInput":
            if name != pname:
                in_names.append(name)
                in_dtypes.append(mybir.dt.np(alloc.dtype))
        elif alloc.kind == "ExternalOutput":
            out_names.append(name)
            shape = tuple(alloc.tensor_shape)
            dtype = mybir.dt.np(alloc.dtype)
            out_avals.append(jax.core.ShapedArray(shape, dtype))
            zero_outs.append(np.zeros(shape, dtype))
    return pname, in_names, in_dtypes, out_names, out_avals, zero_outs


def make_exec(nc):
    """jit callable executing nc's NEFF on one device (args: ins + zero y)."""
    import jax
    from concourse import bass2jax as b2j

    b2j.install_neuronx_cc_hook()
    pname, in_names, in_dtypes, out_names, out_avals, zero_outs = \
        _exec_parts(nc)
    all_names = in_names + out_names
    if pname is not None:
        all_names = all_names + [pname]

    def _bodyfn(*args):
        operands = list(args)
        if pname is not None:
            operands.append(b2j.partition_id_tensor())
        return tuple(b2j._bass_exec_p.bind(
            *operands, out_avals=tuple(out_avals), in_names=tuple(all_names),
            out_names=tuple(out_names), lowering_input_output_aliases=(),
            sim_require_finite=True, sim_require_nnan=True, nc=nc))

    return jax.jit(_bodyfn)


def get_exec():
    """Build (once) a cached single-device jit callable for the NEFF.

    run_bass_via_pjrt re-traces a fresh jit closure on every call; caching
    the compiled callable makes repeat kernel() invocations cheap.
    """
    global _NC, _EXEC
    if _EXEC is not None:
        return _EXEC
    if _NC is None:
        _NC = build_nc()
    fn = make_exec(_NC)
    _, in_names, in_dtypes, _, _, zero_outs = _exec_parts(_NC)
    _EXEC = (fn, in_names, in_dtypes, zero_outs)
    return _EXEC


def prep_inputs(x, w_qkv, w_proj, b_proj):
    bf = ml_dtypes.bfloat16
    return {
        "x": np.ascontiguousarray(
            np.asarray(x, dtype=np.float32).astype(bf)).reshape(B * N, D),
        "w_qkv": np.ascontiguousarray(
            np.asarray(w_qkv, dtype=np.float32).astype(bf)),
        "w_proj": np.ascontiguousarray(
            np.asarray(w_proj, dtype=np.float32).astype(bf)),
        "b_proj": np.ascontiguousarray(np.asarray(b_proj, dtype=np.float32)),
    }


def kernel(x, w_qkv, w_proj, b_proj):
    import jax
    fn, in_names, in_dtypes, zero_outs = get_exec()
    host = prep_inputs(x, w_qkv, w_proj, b_proj)
    args = [np.asarray(host[n_], dtype=dt_) for n_, dt_ in
            zip(in_names, in_dtypes)] + [z.copy() for z in zero_outs]
    outs = fn(*args)
    y = np.asarray(jax.block_until_ready(outs)[0], dtype=np.float32)
    return y.reshape(B, N, D)
